# revision 3
# baseline (speedup 1.0000x reference)
"""Deformable Conv2D (DCNv2-style) on 8 Trainium2 NeuronCores — v2.

Conv-first reformulation (per core, one sample):
  Y_kk = W[:,:,kk] @ x over all spatial positions; out[f,j] = sum_kk sum_c
  w_c[kk,j] * Y_kk[f, p_c(kk,j)].

v2 vs baseline:
  - 2-slot DRAM table TC[r] = [Y(r-65) | Y(r-1)] (512B rows) + elem_step
    overlap gather: one 1KB descriptor at idx r=t+65 with elem_step=256
    fetches rows (r, r+1) = corners (00, 10, 01, 11).  Halves the table
    write traffic and the stage-A matmul count vs the 4-slot layout.
  - Gathers round-robin over 4 SWDGE queues: Q7 descriptor generation for
    queue q runs on core pair (2q, 2q+1), so 4 queues generate in parallel
    (~4x less Pool critical path).
  - Combine via a custom DVE op DUAL_AXPY (out = in0*s0 + in1*s1) and the
    separable bilinear factorization:
      u|v = hy'*[g00|g01] + ly'*[g10|g11]      (one FD=256 op)
      t_b = hxm'*u + lxm'*v                    (one FD=128 op)
    with validity/mask folded into the four f32 per-position scalars.
  - Tap accumulation + output transpose fused into one PE matmul per
    (tap, block): acc[f,q] += t_b.T via matmul(lhsT=t_b, rhs=identity)
    with PSUM accumulation (start at tap 0, stop at tap 8).  No stage C.

Shapes (hardcoded per spec): x (8,128,64,64) f32, offset (8,18,64,64),
mask (8,9,64,64), weight (128,128,3,3), out (8,128,64,64) f32.
"""

import numpy as np
import ml_dtypes
from contextlib import ExitStack

import concourse.bass as bass
import concourse.bacc as bacc
import concourse.tile as tile
from concourse import mybir
from concourse.bass_utils import run_bass_kernel_spmd

B, C, H, W = 8, 128, 64, 64
F = 128
KH = KW = 3
KK = KH * KW
HW = H * W  # 4096
NP = 128
NJB = HW // NP  # 32 j-blocks
NTT = 33
# Table rows carry a 64-row head pad so the slot1 shifted writes (row r-64)
# stay in range, and a zero-filled tail [4288, 4352) for idx+1 edge reads.
TPAD = 64
TROWS = 34 * NP  # 4352 allocated rows per tap
D = 2 * F  # 256 elements (512B) per table row
TBLKK = TROWS * D
XPAD_LO = 65
XPAD = XPAD_LO + NTT * NP + 64

BF16 = mybir.dt.bfloat16
F32 = mybir.dt.float32
I16 = mybir.dt.int16

PERF_EN = True  # 2x perf-mode opt-in for DUAL_AXPY (validated by rel-err check)


def _register_dual_axpy():
    """Register the DUAL_AXPY custom DVE op (out = in0*s0 + in1*s1) into
    concourse.dve_ops' registry so codegen + table-gen find it."""
    from concourse import dve_ops as dvo
    from concourse.dve_spec import C0, C1, Spec, Src0, Src1, lower, _has_src1
    from concourse.dve_uop import DveOpSpec

    name = "DUAL_AXPY_DCN"
    if name in dvo._SUB_OPCODE_FOR_NAME:
        return next(op for op in dvo.OPS if op.name == name)

    spec = Spec(
        body=Src0 * C0 + Src1 * C1,
        reference=lambda in0, in1, s0, s1, imm2: in0.astype(np.float32) * s0
        + in1.astype(np.float32) * s1,
    )
    row = max(dvo._SUB_OPCODE_FOR_NAME.values()) + 1
    assert row < 0x20
    dvo._SUB_OPCODE_FOR_NAME[name] = row
    shas = {}
    for ver in ("v3", "v4"):
        s = DveOpSpec(
            name=name, opcode=row, uops=lower(spec, ver=ver), rd1_en=_has_src1(spec)
        )
        shas[ver] = s.sha(ver)
    op = dvo.DveOp(
        name, spec, subdim=False, uops_sha=shas,
        perf_en={"v3": PERF_EN, "v4": PERF_EN},
    )
    dvo.OPS.append(op)
    dvo.CUSTOM_DVE_SPECS[name] = spec
    return op


DUAL_AXPY = _register_dual_axpy()


def _prep_indices_weights(offset, mask):
    """Per-sample host prep. offset [18,H,W], mask [9,H,W] ->
    idx int16 [128, KK*256], wts f32 [128, KK*4*NJB] with the factorized
    scalars (hy', ly', hxm', lxm') per (kk, j)."""
    off = offset.reshape(KK, 2, H, W)
    dy, dx = off[:, 0], off[:, 1]
    ki, kj = np.meshgrid(np.arange(KH), np.arange(KW), indexing="ij")
    ki = ki.reshape(KK, 1, 1).astype(np.float32)
    kj = kj.reshape(KK, 1, 1).astype(np.float32)
    base_y = (np.arange(H, dtype=np.float32) - 1.0)[None, :, None] + ki
    base_x = (np.arange(W, dtype=np.float32) - 1.0)[None, None, :] + kj
    py = base_y + dy
    px = base_x + dx
    y0 = np.floor(py)
    x0 = np.floor(px)
    ly = (py - y0).astype(np.float32)
    lx = (px - x0).astype(np.float32)
    hy = 1.0 - ly
    hx = 1.0 - lx
    y0i = y0.astype(np.int64)
    x0i = x0.astype(np.int64)

    vy0 = ((y0i >= 0) & (y0i < H)).astype(np.float32)
    vy1 = ((y0i + 1 >= 0) & (y0i + 1 < H)).astype(np.float32)
    vx0 = ((x0i >= 0) & (x0i < W)).astype(np.float32)
    vx1 = ((x0i + 1 >= 0) & (x0i + 1 < W)).astype(np.float32)

    m = mask.reshape(KK, H, W)
    hyv = (hy * vy0).reshape(KK, HW).astype(np.float32)
    lyv = (ly * vy1).reshape(KK, HW).astype(np.float32)
    hxm = (hx * m * vx0).reshape(KK, HW).astype(np.float32)
    lxm = (lx * m * vx1).reshape(KK, HW).astype(np.float32)

    flat = np.clip(y0i * W + x0i + TPAD + 65, TPAD, HW + TPAD + 64).reshape(KK, HW)

    idx_dev = np.zeros((128, KK * 256 + 16), np.int16)
    for kk in range(KK):
        wrapped = flat[kk].astype(np.int16).reshape(256, 16).T  # [16, 256]
        idx_dev[:, kk * 256 : (kk + 1) * 256] = np.tile(wrapped, (8, 1))

    scalars = (hxm, lxm, hyv, lyv)
    wts_dev = np.empty((128, KK * 4 * NJB), np.float32)
    col = 0
    for kk in range(KK):
        for ci in range(4):
            wc = scalars[ci][kk].reshape(NJB, 128)
            wts_dev[:, col : col + NJB] = wc.T
            col += NJB
    return idx_dev, wts_dev


def _split_overfull_waits(nc):
    """This walrus build accepts 1 sync-wait per instruction (2 for EVSEM).
    Move extras onto preceding same-engine NoOps."""
    for f in nc.m.functions:
        for bb in f.blocks:
            new_list = []
            for ins in bb.instructions:
                si = ins.sync_info
                waits = list(si.on_wait) if si and si.on_wait else []
                cap = 2 if isinstance(ins, mybir.InstEventSemaphore) else 1
                if len(waits) > cap:
                    extra, keep = waits[:-cap], waits[-cap:]
                    for k, w in enumerate(extra):
                        nop = mybir.InstNoOp(
                            name=f"{ins.name}_waitsplit{k}",
                            sync_info=mybir.SyncInfo(on_wait=[w], on_update=[]),
                            bass_nofuse=True,
                            engine=ins.engine,
                        )
                        new_list.append(nop)
                        nc.register_instruction(nop, overwrite=True)
                    si.on_wait = keep
                new_list.append(ins)
            bb.instructions[:] = new_list


def _build_nc():
    nc = bacc.Bacc(None, target_bir_lowering=False, debug=False, num_swdge_queues=4)
    x_d = nc.dram_tensor("x", [NP, XPAD], BF16, kind="ExternalInput")
    wt_d = nc.dram_tensor("wt", [NP, KK * F], BF16, kind="ExternalInput")
    idx_d = nc.dram_tensor("idx", [NP, KK * 256 + 16], I16, kind="ExternalInput")
    wts_d = nc.dram_tensor("wts", [NP, KK * 4 * NJB], F32, kind="ExternalInput")
    ident_d = nc.dram_tensor("ident", [NP, NP], BF16, kind="ExternalInput")
    out_d = nc.dram_tensor("out", [NP, HW], F32, kind="ExternalOutput")
    tbl_d = nc.dram_tensor("tbl", [KK, TROWS, D], BF16, kind="Internal")

    with tile.TileContext(nc) as tc, ExitStack() as ctx:
        cpool = ctx.enter_context(tc.tile_pool(name="const", bufs=1))
        tcst_pool = ctx.enter_context(tc.tile_pool(name="tcst", bufs=2))
        gpool = ctx.enter_context(tc.tile_pool(name="gat", bufs=6))
        dpool = ctx.enter_context(tc.tile_pool(name="dmy", bufs=1))
        uvpool = ctx.enter_context(tc.tile_pool(name="uv", bufs=26))
        tbpool = ctx.enter_context(tc.tile_pool(name="tb", bufs=36))
        opool = ctx.enter_context(tc.tile_pool(name="ot", bufs=1))
        psA = ctx.enter_context(tc.tile_pool(name="psA", bufs=2, space="PSUM"))
        psAcc = ctx.enter_context(tc.tile_pool(name="psAcc", bufs=1, space="PSUM"))

        x_sb = cpool.tile([NP, XPAD], BF16)
        wt_sb = cpool.tile([NP, KK * F], BF16)
        idx_sb = cpool.tile([NP, KK * 256 + 16], I16)
        wts_sb = cpool.tile([NP, KK * 4 * NJB], F32)
        id_sb = cpool.tile([NP, NP], BF16)
        zfill = cpool.tile([NP, KK, D], BF16)
        out_sb = opool.tile([NP, HW], F32)

        nc.sync.dma_start(x_sb[:], x_d[:])
        nc.sync.dma_start(wt_sb[:], wt_d[:])
        nc.sync.dma_start(idx_sb[:], idx_d[:])
        nc.sync.dma_start(wts_sb[:], wts_d[:])
        nc.sync.dma_start(id_sb[:], ident_d[:])
        nc.vector.memset(zfill[:], 0.0)

        # zero the tail rows [4224, 4352) of every tap's table (the idx+1
        # edge reads may touch rows whose slot1 no DMA writes)
        nc.sync.dma_start(
            bass.AP(
                tbl_d,
                (HW + 2 * TPAD) * D,
                [[D, NP], [TBLKK, KK], [1, D]],
            ),
            zfill[:],
        )

        # warm up the 4 SWDGE gather queues (first call per queue pays ~10us+
        # of one-time cost; these tiny gathers absorb it off the critical path)
        for q in range(4):
            dmy = dpool.tile([NP, 1, 2 * D], BF16, tag=f"dmy{q}")
            nc.gpsimd.dma_gather(
                out_ap=dmy[:],
                in_ap=bass.AP(x_d, 0, [[D, 64], [1, 2 * D]]),
                idxs_ap=idx_sb[:, KK * 256 : KK * 256 + 8],
                num_idxs=NP,
                num_idxs_reg=NP,
                elem_size=2 * D,
                elem_step=D,
                single_packet=False,
                queue_num=q,
            )

        # ---- Stage A: one matmul per (group, tt); slot1 content equals
        # slot0 shifted by 64 rows, so the staged bytes are DMA'd twice:
        # slot0 at rows TPAD+tt*128, slot1 (row offset F) at rows tt*128.
        # Evictions stage 4 tt-tiles, then one DMA pair (fewer SP issues).
        for g in range(3):
            for t0 in range(0, NTT, 17):
                nq = min(17, NTT - t0)
                tcst = tcst_pool.tile([NP, 17, 3, F], BF16, tag="tcst")
                for tq in range(nq):
                    tt = t0 + tq
                    ps = psA.tile([NP, 512], F32, tag="psA")
                    xoff = XPAD_LO + tt * NP - 65
                    nc.tensor.matmul(
                        ps[:, 0 : 3 * F],
                        x_sb[:, xoff : xoff + NP],
                        wt_sb[:, g * 3 * F : (g + 1) * 3 * F],
                        start=True,
                        stop=True,
                    )
                    src = ps[:, 0 : 3 * F].rearrange("p (k f) -> p k f", k=3)
                    if tt % 2 == 0:
                        nc.scalar.copy(tcst[:, tq], src)
                    else:
                        nc.vector.tensor_copy(tcst[:, tq], src)
                for k in range(3):
                    for base, coloff in (
                        ((TPAD + t0 * NP) * D, 0),
                        (t0 * NP * D, F),
                    ):
                        nc.sync.dma_start(
                            bass.AP(
                                tbl_d,
                                (3 * g + k) * TBLKK + base + coloff,
                                [[D, NP], [NP * D, nq], [1, F]],
                            ),
                            tcst[:, 0:nq, k, :],
                            single_packet=True,
                        )

        # ---- Stage B: gather + DUAL_AXPY combine + PE transpose-accumulate
        for hh in range(2):
            acc = psAcc.tile([NP, 16, NP], F32, tag="acc")
            for kk in range(KK):
                wbase = kk * 4 * NJB
                g_t = gpool.tile([NP, 16, 2 * D], BF16, tag="g_t")
                src = bass.AP(tbl_d, kk * TBLKK, [[D, TROWS - 1], [1, 2 * D]])
                if hh == 0 and kk == 0:
                    # split the pipeline-head gather across all 4 queues so
                    # its descriptor generation runs 4-way parallel
                    for sq in range(4):
                        nc.gpsimd.dma_gather(
                            out_ap=g_t[:, sq * 4 : (sq + 1) * 4, :],
                            in_ap=src,
                            idxs_ap=idx_sb[:, kk * 256 + sq * 32 : kk * 256 + (sq + 1) * 32],
                            num_idxs=HW // 8,
                            num_idxs_reg=HW // 8,
                            elem_size=2 * D,
                            elem_step=D,
                            single_packet=False,
                            queue_num=sq,
                        )
                else:
                    nc.gpsimd.dma_gather(
                        out_ap=g_t[:],
                        in_ap=src,
                        idxs_ap=idx_sb[:, kk * 256 + hh * 128 : kk * 256 + (hh + 1) * 128],
                        num_idxs=HW // 2,
                        num_idxs_reg=HW // 2,
                        elem_size=2 * D,
                        elem_step=D,
                        single_packet=False,
                        queue_num=(hh * KK + kk) % 4,
                    )
                for ii in range(16):
                    i = hh * 16 + ii
                    s_hxm = wts_sb[:, wbase + i : wbase + i + 1]
                    s_lxm = wts_sb[:, wbase + NJB + i : wbase + NJB + i + 1]
                    s_hy = wts_sb[:, wbase + 2 * NJB + i : wbase + 2 * NJB + i + 1]
                    s_ly = wts_sb[:, wbase + 3 * NJB + i : wbase + 3 * NJB + i + 1]
                    # slots (00, 10, 01, 11): x-lerp the [g00|g10] / [g01|g11]
                    # halves first (contiguous 2D APs), then y-lerp u|v.
                    u_v = uvpool.tile([NP, 2 * F], BF16, tag="uv")
                    nc.vector._custom_dve(
                        DUAL_AXPY,
                        out=u_v[:],
                        in0=g_t[:, ii, 0 : 2 * F],
                        in1=g_t[:, ii, 2 * F : 4 * F],
                        s0=s_hxm,
                        s1=s_lxm,
                    )
                    # start=True zeroes the ENTIRE 2KB psum bank, so only the
                    # first matmul of each bank's first block (ii%4==0) at
                    # kk==0 may set it; everything else accumulates.
                    first = kk == 0 and ii % 4 == 0
                    last = kk == KK - 1
                    if (ii + kk) % 16 < 9:
                        # H-block: y-lerp as two ACT muls + two PE accumulates
                        t_b = tbpool.tile([NP, 2, NP], BF16, tag="tb2")
                        nc.scalar.mul(t_b[:, 0, :], u_v[:, 0:F], s_hy)
                        nc.scalar.mul(t_b[:, 1, :], u_v[:, F : 2 * F], s_ly)
                        nc.tensor.matmul(
                            acc[:, ii, :], t_b[:, 0, :], id_sb[:],
                            start=first, stop=False, skip_group_check=True,
                        )
                        nc.tensor.matmul(
                            acc[:, ii, :], t_b[:, 1, :], id_sb[:],
                            start=False, stop=last, skip_group_check=True,
                        )
                    else:
                        # D-block: y-lerp on DVE + one PE accumulate
                        t_b = tbpool.tile([NP, NP], BF16, tag="tb")
                        nc.vector._custom_dve(
                            DUAL_AXPY,
                            out=t_b[:],
                            in0=u_v[:, 0:F],
                            in1=u_v[:, F : 2 * F],
                            s0=s_hy,
                            s1=s_ly,
                        )
                        nc.tensor.matmul(
                            acc[:, ii, :], t_b[:], id_sb[:],
                            start=first, stop=last, skip_group_check=True,
                        )
            nc.scalar.copy(out_sb[:, hh * 2048 : (hh + 1) * 2048], acc[:])
            nc.sync.dma_start(
                bass.AP(out_d, hh * 2048, [[HW, NP], [1, 2048]]),
                out_sb[:, hh * 2048 : (hh + 1) * 2048],
            )

    nc.compile()
    _split_overfull_waits(nc)
    return nc


_NC_CACHE = {}


def _get_nc():
    if "nc" not in _NC_CACHE:
        _NC_CACHE["nc"] = _build_nc()
    return _NC_CACHE["nc"]


def _prep_x(xb):
    """x [C,H,W] f32 -> padded bf16 [128, XPAD]."""
    xp = np.zeros((C, XPAD), ml_dtypes.bfloat16)
    xp[:, XPAD_LO : XPAD_LO + HW] = xb.reshape(C, HW).astype(ml_dtypes.bfloat16)
    return xp


def kernel(x, offset, mask, weight, **run_kwargs):
    x = np.asarray(x, np.float32)
    offset = np.asarray(offset, np.float32)
    mask = np.asarray(mask, np.float32)
    weight = np.asarray(weight, np.float32)

    wt = np.transpose(weight.reshape(F, C, KK), (1, 2, 0)).reshape(C, KK * F)
    wt = np.ascontiguousarray(wt).astype(ml_dtypes.bfloat16)
    ident = np.eye(NP, dtype=ml_dtypes.bfloat16)

    in_maps = []
    for b in range(B):
        idx_dev, wts_dev = _prep_indices_weights(offset[b], mask[b])
        in_maps.append(
            {
                "x": _prep_x(x[b]),
                "wt": wt,
                "idx": idx_dev,
                "wts": wts_dev,
                "ident": ident,
            }
        )

    nc = _get_nc()
    res = run_bass_kernel_spmd(nc, in_maps, core_ids=list(range(8)), **run_kwargs)
    out = np.stack([np.asarray(res.results[b]["out"]).reshape(F, H, W) for b in range(B)])
    if run_kwargs:
        kernel.last_results = res
    return out
